# revision 5
# baseline (speedup 1.0000x reference)
"""Contextual patches score kernel for Trainium2 (8 NeuronCores).

Computes, per sample i:
    fs = f[i, :, ::2, ::2]; bs = b[i, :, ::2, ::2]          # [64, 80, 80]
    w  = 3x3 patches of bs (SAME, stride 1)                  # [6400, 64, 3, 3]
    wn = w / max(||w||_2, 1e-4)
    y[i] = conv(fs, wn, SAME)                                # [6400, 80, 80]

Implementation: y[l, p] = (w_l . f_patch_p) * inv_norm_l is a
[6400, 576] x [576, 6400] matmul per sample.  Sharding: 8 cores =
2 samples x 4 spatial-row quarters; each core computes [6400, 1600].
K = 576 = 64 channels x 9 taps, packed as 5 chunks of 128 partitions
(tap pairs stacked; last chunk zero-padded).  Operands are built once
in SBUF by copying shifted windows out of zero-padded images (a
row-shifted replica of each image lives in partitions 64-127 so a tap
pair is a single lane-aligned copy).  Patch normalization is applied
as a per-output-row scale after PSUM accumulation.
"""

import numpy as np

import concourse.bass as bass
import concourse.mybir as mybir
import concourse.tile as tile
from concourse.bass_utils import run_bass_kernel_spmd

F32 = mybir.dt.float32
F32R = mybir.dt.float32r
AF = mybir.ActivationFunctionType

C = 64            # channels
H = W = 80        # downsampled spatial size
L = H * W         # 6400 patches per sample
QROWS = 20        # output rows handled per core
POS = QROWS * W   # 1600 output positions per core
NTILE = 400       # matmul moving free dim (5 rows x 80)
NT = POS // NTILE         # 4 n-tiles
MT = L // 128             # 50 m-tiles
HALF_MT = MT // 2         # 25 (lhsT is split in two halves for pipelining)
NCHUNK = 5                # K chunks: 4 full tap pairs + 1 half (tap 8)
EPS = 1e-4

# chunk -> ((kh, kw) for partitions 0:64, (kh, kw) for partitions 64:128)
# The replica half of each padded image is shifted up one row, so a
# (kh, kw) / (kh+1, kw') pair reads with a single AP offset per half.
_CHUNK_TAPS = [
    ((0, 0), (1, 0)),
    ((0, 1), (1, 1)),
    ((0, 2), (1, 2)),
    ((2, 0), (2, 1)),
    ((2, 2), None),
]


def _win(img, kh, kw, nrows):
    """[*, nrows, 80] shifted window of a padded [*, rows, 82] image tile."""
    return img[:, kh:kh + nrows, kw:kw + W]


def _copy_chunks(nc, dst3, img, nrows):
    """Fill dst3 [128, 5, nrows*80] with the 5 K-chunks of im2col windows.

    img: [128, nrows+2, 82] padded image; partitions 64:128 hold the
    same image shifted up one row (img2[c, r, x] = img1[c, r+1, x]).
    """
    def dst(j, p0, p1):
        return dst3[p0:p1, j, :].rearrange("p (y x) -> p y x", x=W)

    for j in range(3):
        (kh, kw), _ = _CHUNK_TAPS[j]
        nc.vector.tensor_copy(dst(j, 0, 128), _win(img, kh, kw, nrows))
    # chunk 3: tap (2,0) from base half, tap (2,1) via replica (kh-1 index)
    nc.vector.tensor_copy(dst(3, 0, 64), _win(img[0:64], 2, 0, nrows))
    nc.vector.tensor_copy(dst(3, 64, 128), _win(img[64:128], 1, 1, nrows))
    # chunk 4: tap (2,2); upper partitions stay zero (memset elsewhere)
    nc.vector.tensor_copy(dst(4, 0, 64), _win(img[0:64], 2, 2, nrows))


def build_nc():
    nc = bass.Bass(target_bir_lowering=False)
    fs_d = nc.dram_tensor("fs_pad", [C, QROWS + 2, 82], F32, kind="ExternalInput")
    bs_d = nc.dram_tensor("bs_pad", [C, 82, 82], F32, kind="ExternalInput")
    y_d = nc.dram_tensor("y", [L, POS], F32, kind="ExternalOutput")

    with tile.TileContext(nc) as tc:
        with (
            tc.tile_pool(name="big", bufs=1) as big,
            tc.tile_pool(name="pad", bufs=2) as padp,
            tc.tile_pool(name="sq", bufs=2) as sqp,
            tc.tile_pool(name="inv", bufs=4) as invp,
            tc.tile_pool(name="outp", bufs=3) as outp,
            tc.tile_pool(name="ps", bufs=4, space="PSUM") as psp,
            tc.tile_pool(name="pss", bufs=2, space="PSUM") as pssp,
        ):
            ones = big.tile([128, 2], F32R, tag="ones")
            nc.vector.memset(ones[:].bitcast(F32), 1.0)

            # f image quarter + row-shifted replica in partitions 64:128
            fpad = big.tile([128, QROWS + 2, 82], F32, tag="fpad")
            nc.sync.dma_start(fpad[0:64], fs_d[:])
            nc.sync.dma_start(fpad[64:128, 0:QROWS + 1], fs_d[:, 1:QROWS + 2])

            # rhs: im2col of the f quarter, [128, 5, 1600]
            rhs = big.tile([128, NCHUNK, POS], F32R, tag="rhs")
            nc.vector.memset(rhs[64:128, 4, :].bitcast(F32), 0.0)
            _copy_chunks(nc, rhs, fpad, QROWS)

            # lhsT: b patches (transposed weights), two halves of
            # [128, 5, 3200] so early matmuls do not wait on the full build
            lhsT = []
            for h in range(2):
                lh = big.tile([128, NCHUNK, HALF_MT * 128], F32R, tag=f"lhsT{h}")
                nc.vector.memset(lh[64:128, 4, :].bitcast(F32), 0.0)
                for qq in range(2):
                    qi = 2 * h + qq
                    bt = padp.tile([128, QROWS + 2, 82], F32, tag="bpad")
                    nc.sync.dma_start(bt[0:64], bs_d[:, 20 * qi:20 * qi + 22])
                    nc.sync.dma_start(
                        bt[64:128, 0:QROWS + 1],
                        bs_d[:, 20 * qi + 1:20 * qi + 22],
                    )
                    _copy_chunks(
                        nc, lh[:, :, qq * POS:(qq + 1) * POS], bt, QROWS
                    )
                lhsT.append(lh)

            for m in range(MT):
                h, ml = divmod(m, HALF_MT)
                msl = slice(ml * 128, (ml + 1) * 128)

                # inv_norm for these 128 patches: ones-matmul over squares
                sq = sqp.tile([128, NCHUNK, 128], F32R, tag="sq")
                nc.scalar.activation(sq[:], lhsT[h][:, :, msl], AF.Square)
                ps_s = pssp.tile([128, 2], F32, tag="pss")
                for j in range(NCHUNK):
                    nc.tensor.matmul(
                        ps_s[:],
                        lhsT=sq[:, j, :],
                        rhs=ones[:],
                        start=(j == 0),
                        stop=(j == NCHUNK - 1),
                    )
                inv = invp.tile([128, 1], F32, tag="inv")
                nc.scalar.activation(inv[:], ps_s[:, 0:1], AF.Sqrt)
                nc.vector.tensor_scalar(
                    inv[:], inv[:], EPS, None, mybir.AluOpType.max
                )
                nc.vector.reciprocal(inv[:], inv[:])

                for nt in range(NT):
                    ps = psp.tile([128, NTILE], F32, tag="ps")
                    for j in range(NCHUNK):
                        nc.tensor.matmul(
                            ps[:],
                            lhsT=lhsT[h][:, j, msl],
                            rhs=rhs[:, j, nt * NTILE:(nt + 1) * NTILE],
                            start=(j == 0),
                            stop=(j == NCHUNK - 1),
                        )
                    ot = outp.tile([128, NTILE], F32, tag="ot")
                    nc.vector.tensor_scalar_mul(ot[:], ps[:], inv[:])
                    nc.sync.dma_start(
                        y_d[m * 128:(m + 1) * 128, nt * NTILE:(nt + 1) * NTILE],
                        ot[:],
                    )
    return nc


def _split_multiwaits(nc, maxw=1):
    """Walrus (this build) accepts at most one sync-wait per instruction.

    Tile's kernel-tail drain carries one wait per active logical proc, so
    hoist excess waits onto same-engine NoOps inserted right before the
    offending instruction (engine executes them in order -> identical
    blocking semantics)."""
    n = 0
    for fn in nc.m.functions:
        for blk in fn.blocks:
            insts = list(blk.instructions)
            new, changed = [], False
            for ins in insts:
                si = ins.sync_info
                if si is not None and len(si.on_wait) > maxw:
                    extra, keep = si.on_wait[:-maxw], si.on_wait[-maxw:]
                    k = 0
                    while extra:
                        chunk, extra = extra[:maxw], extra[maxw:]
                        new.append(mybir.InstNoOp(
                            name=f"{ins.name}-ws{k}",
                            engine=ins.engine,
                            bass_nofuse=True,
                            sync_info=mybir.SyncInfo(
                                on_wait=list(chunk), on_update=[]
                            ),
                        ))
                        k += 1
                        n += 1
                    ins.sync_info = mybir.SyncInfo(
                        on_wait=list(keep), on_update=list(si.on_update)
                    )
                    changed = True
                new.append(ins)
            if changed:
                blk.instructions = new
    return n


_CACHE = {}


def _get_nc():
    if "nc" not in _CACHE:
        nc = build_nc()
        _split_multiwaits(nc)
        _CACHE["nc"] = nc
    return _CACHE["nc"]


def make_in_maps(f, b):
    f = np.asarray(f, dtype=np.float32)
    b = np.asarray(b, dtype=np.float32)
    n_samples = f.shape[0]
    fs = f[:, :, ::2, ::2]
    bs = b[:, :, ::2, ::2]
    fpad = np.zeros((n_samples, C, 82, 82), np.float32)
    fpad[:, :, 1:81, 1:81] = fs
    bpad = np.zeros((n_samples, C, 82, 82), np.float32)
    bpad[:, :, 1:81, 1:81] = bs
    in_maps = []
    for c in range(8):
        n, q = divmod(c, 4)
        in_maps.append({
            "fs_pad": np.ascontiguousarray(fpad[n, :, 20 * q:20 * q + 22, :]),
            "bs_pad": np.ascontiguousarray(bpad[n]),
        })
    return in_maps


def assemble(results, n_samples=2):
    out = np.empty((n_samples, L, H, W), np.float32)
    for c in range(8):
        n, q = divmod(c, 4)
        out[n, :, 20 * q:20 * q + 20, :] = results[c]["y"].reshape(L, QROWS, W)
    return out


def run(f, b, **kw):
    res = run_bass_kernel_spmd(_get_nc(), make_in_maps(f, b), list(range(8)), **kw)
    return assemble(res.results, np.asarray(f).shape[0]), res


def kernel(f, b):
    out, _ = run(f, b)
    return out
